# revision 6
# baseline (speedup 1.0000x reference)
"""Trainium2 Bass kernel for nn_ConceptIntergation (histogram_binning).

Reference computation:
    counts[b,s,n] = sum_k one_hot(concepts[b,s,k], 129)[..., n]  (n < 128; 128 = padding)
    out[b,s,n,d]  = counts[b,s,n] * emb_table[n,d]

Strategy (data-parallel over batch, 8 cores):
  - Each core handles B_LOC=8 batches -> 1600 (b,s) rows, output shard
    [1600, 128*64] f32 (~52 MB). The kernel is HBM-write bound; the whole
    design keeps the 16 SDMA store engines saturated from ~4us to the end.
  - Rows are processed in 128-row blocks (rows on partitions). Histogram via
    iota-compare on DVE (tensor_scalar is_equal + scalar_tensor_tensor
    accumulate), then broadcast tensor_tensor multiplies produce
    [128, 2048] chunks = counts[:,n] * emb[n,d]; each chunk is a 1 MB DMA
    store (contiguous 8 KB per partition).
  - HBM traffic is (almost) stores only: the 128-partition emb replica is
    NOT loaded from HBM (that would be 4 MB = ~10 us of the ~140 us HBM
    budget). Chunk 0 (1 MB) loads during the otherwise-idle ramp; chunks
    1..3 are replicated on-chip by the idle TensorEngine (bf16
    ones[1,128]^T @ emb1[1,512] outer products -> PSUM -> ScalarE copy to
    SBUF). bf16 rounding of emb (rel err ~2^-8) is far inside the 2e-2
    tolerance and only affects chunks 1..3.
  - SDMA engine 15 runs ~15-20% slower than the others in every profiled
    run (known trn2 behavior; engine 7 intermittently too). Engine load is
    set by the SBUF partition->port swizzle (port = bits[4:2]<<1 | bit[6]);
    engine 15 serves partitions {92-95,124-127}, engine 7 {76-79,108-111}.
    Row placement (1600 = 12*128 + 64) is chosen so those partitions carry
    11-12 rows while the rest carry 12-13:
      blocks 0..10  full 128 rows, row = 128j + p
      block 11      compute on all 128 partitions, but stores skip engine
                    15's partitions: [0:92] -> rows 1408+p and [96:124] ->
                    rows 1500+(p-96); the 8 displaced rows move to the
                    remainder block
      remainder     compute on partitions 0..71 (one DVE op costs the same
                    for 72 as for 128 partitions): [0:64] -> rows 1536+p,
                    [64:72] -> rows 1528+(p-64)
    Per-engine rows: even ports 104, e1/e3 100, e5/e9/e11/e13 96, e7 96,
    e15 88 -- ~equal finish times given e15's measured ~20.4 GB/s.
"""

import numpy as np
import ml_dtypes

import concourse.bass as bass
import concourse.mybir as mybir
from concourse import bacc
from concourse.tile import TileContext
from concourse.bass_utils import run_bass_kernel_spmd

B, S, K = 64, 200, 4
N, D = 128, 64
ND = N * D                      # 8192
NCORES = 8
B_LOC = B // NCORES             # 8
ROWS = B_LOC * S                # 1600 (b,s) rows per core
P = 128
NBLK = 13                       # 11 full + block 11 (split stores) + remainder

CH = 4                          # emb/mul/store chunks per block
CW = ND // CH                   # 2048 cols per chunk (= 32 n-rows), 1 MB stores
NCH = N // CH                   # 32 n-rows per chunk
MMW = 512                       # matmul moving-dim width (HW max)

# per-block (partition window, compute width, store segments (p0, p1, row0))
BLOCKS = []
for j in range(11):
    BLOCKS.append((P, [(0, P, 128 * j)]))
BLOCKS.append((P, [(0, 92, 1408), (96, 124, 1500 - 96)]))
BLOCKS.append((72, [(0, 64, 1536), (64, 72, 1528 - 64)]))

_NC_CACHE = {}


def _build_nc():
    nc = bacc.Bacc()
    idx = nc.declare_dram_parameter("idx", [P, NBLK * K], mybir.dt.float32, isOutput=False)
    embone = nc.declare_dram_parameter("embone", [1, ND], mybir.dt.bfloat16, isOutput=False)
    embmini = nc.declare_dram_parameter("embmini", [P, CW], mybir.dt.float32, isOutput=False)
    iota = nc.declare_dram_parameter("iota", [P, N], mybir.dt.float32, isOutput=False)
    out = nc.declare_dram_parameter("out", [ROWS, ND], mybir.dt.float32, isOutput=True)

    with TileContext(nc) as tc:
        with (
            tc.tile_pool(name="const", bufs=1) as cpool,
            tc.tile_pool(name="counts", bufs=NBLK) as hpool,
            tc.tile_pool(name="work", bufs=12) as wpool,
            tc.psum_pool(name="psum", bufs=4) as ppool,
        ):
            # emb row first: it feeds the TensorE broadcast of chunks 1..3
            emb1_sb = cpool.tile([1, ND], mybir.dt.bfloat16)
            nc.sync.dma_start(out=emb1_sb, in_=embone[:, :])
            # then the histogram inputs
            iota_sb = cpool.tile([P, N], mybir.dt.float32)
            nc.sync.dma_start(out=iota_sb, in_=iota[:, :])
            idx_sb = cpool.tile([P, NBLK * K], mybir.dt.float32)
            nc.sync.dma_start(out=idx_sb, in_=idx[:, :])
            # chunk 0 of the 128-partition replica rides the idle ramp (1 MB)
            emb_sb = cpool.tile([P, ND], mybir.dt.float32)
            nc.sync.dma_start(out=emb_sb[:, 0:CW], in_=embmini[:, :])

            ones_sb = cpool.tile([1, P], mybir.dt.bfloat16)
            nc.vector.memset(ones_sb, 1.0)

            # chunks 1..3 of the emb replica: outer-product broadcast on the
            # (otherwise idle) TensorEngine, drained PSUM->SBUF by ScalarE.
            for c in range(1, CH):
                for s in range(CW // MMW):
                    col = c * CW + s * MMW
                    pt = ppool.tile([P, MMW], mybir.dt.float32, tag="pt")
                    nc.tensor.matmul(
                        pt[:, :],
                        lhsT=ones_sb[:, :],
                        rhs=emb1_sb[:, col : col + MMW],
                        start=True,
                        stop=True,
                    )
                    nc.scalar.copy(out=emb_sb[:, col : col + MMW], in_=pt[:, :])

            def emit_hist(j, counts):
                pj = BLOCKS[j][0]
                nc.vector.tensor_scalar(
                    out=counts[:pj],
                    in0=iota_sb[:pj],
                    scalar1=idx_sb[:pj, j * K : j * K + 1],
                    scalar2=None,
                    op0=mybir.AluOpType.is_equal,
                )
                for k in range(1, K):
                    nc.vector.scalar_tensor_tensor(
                        out=counts[:pj],
                        in0=iota_sb[:pj],
                        scalar=idx_sb[:pj, j * K + k : j * K + k + 1],
                        in1=counts[:pj],
                        op0=mybir.AluOpType.is_equal,
                        op1=mybir.AluOpType.add,
                    )

            def emit_mul(j, c, counts, split=1):
                pj, segs = BLOCKS[j]
                ot = wpool.tile([P, CW], mybir.dt.float32, tag="ot")
                w = CW // split
                nw = NCH // split
                for s in range(split):
                    nc.vector.tensor_tensor(
                        out=ot[:pj, s * w : (s + 1) * w].rearrange(
                            "p (n d) -> p n d", d=D
                        ),
                        in0=counts[
                            :pj, c * NCH + s * nw : c * NCH + (s + 1) * nw, None
                        ].broadcast_to([pj, nw, D]),
                        in1=emb_sb[:pj, c * CW + s * w : c * CW + (s + 1) * w].rearrange(
                            "p (n d) -> p n d", d=D
                        ),
                        op=mybir.AluOpType.mult,
                    )
                    for p0, p1, r0 in segs:
                        nc.sync.dma_start(
                            out=out[
                                r0 + p0 : r0 + p1, c * CW + s * w : c * CW + (s + 1) * w
                            ],
                            in_=ot[p0:p1, s * w : (s + 1) * w],
                        )

            # chunk-major: the c=0 stripe (gated only on the 1 MB mini load)
            # runs first; histograms are interleaved into it. Block 0 is
            # split into 512-col pieces so the first store issues ASAP.
            counts_tiles = [None] * NBLK
            for j in range(NBLK):
                counts = hpool.tile([P, N], mybir.dt.float32, tag="counts")
                counts_tiles[j] = counts
                emit_hist(j, counts)
                emit_mul(j, 0, counts, split=4 if j == 0 else 1)
            for c in range(1, CH):
                for j in range(NBLK):
                    emit_mul(j, c, counts_tiles[j])

    nc.finalize()
    return nc


def _get_nc():
    if "nc" not in _NC_CACHE:
        _NC_CACHE["nc"] = _build_nc()
    return _NC_CACHE["nc"]


def _prepare_in_maps(concepts, emb_table):
    concepts = np.asarray(concepts)
    emb = np.ascontiguousarray(np.asarray(emb_table, dtype=np.float32).reshape(1, ND))

    # per-core index shards laid out [P, NBLK*K] following BLOCKS row maps
    conc = concepts.reshape(NCORES, ROWS, K).astype(np.float32)
    idx_dev = np.full((NCORES, P, NBLK * K), float(N), dtype=np.float32)
    for j, (pj, segs) in enumerate(BLOCKS):
        for p0, p1, r0 in segs:
            for p in range(p0, p1):
                idx_dev[:, p, j * K : (j + 1) * K] = conc[:, r0 + p]
    idx_dev = np.ascontiguousarray(idx_dev)

    iota = np.ascontiguousarray(
        np.broadcast_to(np.arange(N, dtype=np.float32), (P, N))
    )
    embmini = np.ascontiguousarray(np.broadcast_to(emb[:, :CW], (P, CW)))
    embone = np.ascontiguousarray(emb.astype(ml_dtypes.bfloat16))
    return [
        {"idx": idx_dev[i], "embone": embone, "embmini": embmini, "iota": iota}
        for i in range(NCORES)
    ]


def _run(concepts, emb_table, **spmd_kwargs):
    nc = _get_nc()
    in_maps = _prepare_in_maps(concepts, emb_table)
    res = run_bass_kernel_spmd(nc, in_maps, core_ids=list(range(NCORES)), **spmd_kwargs)
    out = np.concatenate(
        [res.results[i]["out"].reshape(B_LOC, S, N, D) for i in range(NCORES)],
        axis=0,
    )
    return out, res


def kernel(concepts, emb_table):
    out, _ = _run(concepts, emb_table)
    return out


# revision 7
# speedup vs baseline: 1.1715x; 1.1715x over previous
"""Trainium2 Bass kernel for nn_ConceptIntergation (histogram_binning).

Reference computation:
    counts[b,s,n] = sum_k one_hot(concepts[b,s,k], 129)[..., n]  (n < 128; 128 = padding)
    out[b,s,n,d]  = counts[b,s,n] * emb_table[n,d]

Strategy (data-parallel over batch, 8 cores):
  - Each core handles B_LOC=8 batches -> 1600 (b,s) rows, output shard
    [1600, 128*64] f32 (~52 MB). The kernel is HBM-write bound; the design
    keeps the 16 SDMA store engines saturated from ~6us to the end.
  - Rows are processed in 128-row blocks (rows on partitions; remainder 64
    rows on partitions 0..63 — store descriptors are dealt evenly across
    the 16 SDMA engines only for partition counts 128/64 at base 0, so
    only those shapes are used). Histogram via iota-compare on DVE
    (tensor_scalar is_equal + scalar_tensor_tensor accumulate), then
    broadcast tensor_tensor multiplies produce [128, 2048] chunks =
    counts[:,n] * emb[n,d]; each chunk is a 1 MB DMA store (contiguous
    8 KB per partition).
  - HBM traffic is stores only: the 128-partition emb replica is NOT
    loaded from HBM (a 4 MB read = ~10 us of the ~140 us HBM budget; even
    1 MB = ~2.5 us). The whole replica is built on-chip by the otherwise
    idle TensorEngine: bf16 ones[1,128]^T @ emb1[1,512] outer products
    into PSUM, drained to SBUF by the otherwise idle ScalarE. Only a
    16 KB bf16 emb row is read from HBM. bf16 rounding of emb gives rel
    err ~2^-8, far inside the 2e-2 tolerance.
  - Block 0's chunk-0 multiply and store are split into 512-col pieces so
    the first store enters the queue as early as possible.
"""

import numpy as np
import ml_dtypes

import concourse.bass as bass
import concourse.mybir as mybir
from concourse import bacc
from concourse.tile import TileContext
from concourse.bass_utils import run_bass_kernel_spmd

B, S, K = 64, 200, 4
N, D = 128, 64
ND = N * D                      # 8192
NCORES = 8
B_LOC = B // NCORES             # 8
ROWS = B_LOC * S                # 1600 (b,s) rows per core
P = 128
NBLK = (ROWS + P - 1) // P      # 13 (12 full + 1 of 64 rows)

CH = 4                          # emb/mul/store chunks per block
CW = ND // CH                   # 2048 cols per chunk (= 32 n-rows), 1 MB stores
NCH = N // CH                   # 32 n-rows per chunk
MMW = 512                       # matmul moving-dim width (HW max)

_NC_CACHE = {}


def _build_nc():
    nc = bacc.Bacc()
    idx = nc.declare_dram_parameter("idx", [P, NBLK * K], mybir.dt.float32, isOutput=False)
    embone = nc.declare_dram_parameter("embone", [1, ND], mybir.dt.bfloat16, isOutput=False)
    iota = nc.declare_dram_parameter("iota", [P, N], mybir.dt.float32, isOutput=False)
    out = nc.declare_dram_parameter("out", [ROWS, ND], mybir.dt.float32, isOutput=True)

    with TileContext(nc) as tc:
        with (
            tc.tile_pool(name="const", bufs=1) as cpool,
            tc.tile_pool(name="counts", bufs=NBLK) as hpool,
            tc.tile_pool(name="work", bufs=12) as wpool,
            tc.psum_pool(name="psum", bufs=4) as ppool,
        ):
            # emb row first: it feeds the TensorE broadcast of all chunks
            emb1_sb = cpool.tile([1, ND], mybir.dt.bfloat16)
            nc.sync.dma_start(out=emb1_sb, in_=embone[:, :])
            # then the histogram inputs
            iota_sb = cpool.tile([P, N], mybir.dt.float32)
            nc.sync.dma_start(out=iota_sb, in_=iota[:, :])
            idx_sb = cpool.tile([P, NBLK * K], mybir.dt.float32)
            nc.sync.dma_start(out=idx_sb, in_=idx[:, :])

            ones_sb = cpool.tile([1, P], mybir.dt.bfloat16)
            nc.vector.memset(ones_sb, 1.0)

            # 128-partition emb replica built by TensorE outer products,
            # drained PSUM->SBUF by ScalarE. Chunk 0 first: the first
            # multiplies gate on its first 512-col slices.
            emb_sb = cpool.tile([P, ND], mybir.dt.float32)
            for c in range(CH):
                for s in range(CW // MMW):
                    col = c * CW + s * MMW
                    pt = ppool.tile([P, MMW], mybir.dt.float32, tag="pt")
                    nc.tensor.matmul(
                        pt[:, :],
                        lhsT=ones_sb[:, :],
                        rhs=emb1_sb[:, col : col + MMW],
                        start=True,
                        stop=True,
                    )
                    nc.scalar.copy(out=emb_sb[:, col : col + MMW], in_=pt[:, :])

            def emit_hist(j, counts, pj):
                nc.vector.tensor_scalar(
                    out=counts[:pj],
                    in0=iota_sb[:pj],
                    scalar1=idx_sb[:pj, j * K : j * K + 1],
                    scalar2=None,
                    op0=mybir.AluOpType.is_equal,
                )
                for k in range(1, K):
                    nc.vector.scalar_tensor_tensor(
                        out=counts[:pj],
                        in0=iota_sb[:pj],
                        scalar=idx_sb[:pj, j * K + k : j * K + k + 1],
                        in1=counts[:pj],
                        op0=mybir.AluOpType.is_equal,
                        op1=mybir.AluOpType.add,
                    )

            def emit_mul(j, c, counts, pj, split=1):
                ot = wpool.tile([P, CW], mybir.dt.float32, tag="ot")
                w = CW // split
                nw = NCH // split
                for s in range(split):
                    nc.vector.tensor_tensor(
                        out=ot[:pj, s * w : (s + 1) * w].rearrange(
                            "p (n d) -> p n d", d=D
                        ),
                        in0=counts[
                            :pj, c * NCH + s * nw : c * NCH + (s + 1) * nw, None
                        ].broadcast_to([pj, nw, D]),
                        in1=emb_sb[:pj, c * CW + s * w : c * CW + (s + 1) * w].rearrange(
                            "p (n d) -> p n d", d=D
                        ),
                        op=mybir.AluOpType.mult,
                    )
                    nc.sync.dma_start(
                        out=out[
                            j * P : j * P + pj, c * CW + s * w : c * CW + (s + 1) * w
                        ],
                        in_=ot[:pj, s * w : (s + 1) * w],
                    )

            # chunk-major: the c=0 stripe runs first with histograms
            # interleaved; chunks 1..3 follow once TensorE has replicated
            # their emb columns (done by ~16us, needed from ~35us).
            counts_tiles = [None] * NBLK
            for j in range(NBLK):
                pj = min(P, ROWS - j * P)
                counts = hpool.tile([P, N], mybir.dt.float32, tag="counts")
                counts_tiles[j] = counts
                emit_hist(j, counts, pj)
                emit_mul(j, 0, counts, pj, split=4 if j == 0 else 1)
            for c in range(1, CH):
                for j in range(NBLK):
                    pj = min(P, ROWS - j * P)
                    emit_mul(j, c, counts_tiles[j], pj)

    nc.finalize()
    return nc


def _get_nc():
    if "nc" not in _NC_CACHE:
        _NC_CACHE["nc"] = _build_nc()
    return _NC_CACHE["nc"]


def _prepare_in_maps(concepts, emb_table):
    concepts = np.asarray(concepts)
    emb = np.ascontiguousarray(np.asarray(emb_table, dtype=np.float32).reshape(1, ND))

    # per-core index shards, padded to NBLK*P rows, laid out [P, NBLK*K]
    conc = concepts.reshape(NCORES, ROWS, K).astype(np.float32)
    idx_pad = np.full((NCORES, NBLK * P, K), float(N), dtype=np.float32)
    idx_pad[:, :ROWS] = conc
    # [core, NBLK, P, K] -> [core, P, NBLK*K]
    idx_dev = np.ascontiguousarray(
        idx_pad.reshape(NCORES, NBLK, P, K).transpose(0, 2, 1, 3).reshape(NCORES, P, NBLK * K)
    )

    iota = np.ascontiguousarray(
        np.broadcast_to(np.arange(N, dtype=np.float32), (P, N))
    )
    embone = np.ascontiguousarray(emb.astype(ml_dtypes.bfloat16))
    return [
        {"idx": idx_dev[i], "embone": embone, "iota": iota}
        for i in range(NCORES)
    ]


def _run(concepts, emb_table, **spmd_kwargs):
    nc = _get_nc()
    in_maps = _prepare_in_maps(concepts, emb_table)
    res = run_bass_kernel_spmd(nc, in_maps, core_ids=list(range(NCORES)), **spmd_kwargs)
    out = np.concatenate(
        [res.results[i]["out"].reshape(B_LOC, S, N, D) for i in range(NCORES)],
        axis=0,
    )
    return out, res


def kernel(concepts, emb_table):
    out, _ = _run(concepts, emb_table)
    return out


# revision 9
# speedup vs baseline: 1.3120x; 1.1199x over previous
"""Trainium2 Bass kernel for nn_ConceptIntergation (histogram_binning).

Reference computation:
    counts[b,s,n] = sum_k one_hot(concepts[b,s,k], 129)[..., n]  (n < 128; 128 = padding)
    out[b,s,n,d]  = counts[b,s,n] * emb_table[n,d]

Strategy (data-parallel over batch, 8 cores):
  - Each core handles B_LOC=8 batches -> 1600 (b,s) rows, output shard
    [1600, 128*64] f32 (~52 MB). The kernel is HBM-write bound; the design
    keeps the 16 SDMA store engines saturated from ~6us to the end.
  - Rows are processed in 128-row blocks (rows on partitions; remainder 64
    rows on partitions 0..63 — store descriptors are dealt evenly across
    the 16 SDMA engines only for partition counts 128/64 at base 0, so
    only those shapes are used). Histogram via iota-compare on DVE
    (tensor_scalar is_equal + scalar_tensor_tensor accumulate), then
    broadcast tensor_tensor multiplies produce [128, 2048] chunks =
    counts[:,n] * emb[n,d]; each chunk is a 1 MB DMA store (contiguous
    8 KB per partition).
  - HBM traffic is stores only: the 128-partition emb replica is NOT
    loaded from HBM (a 4 MB read = ~10 us of the ~140 us HBM budget; even
    1 MB = ~2.5 us). The whole replica is built on-chip by the otherwise
    idle TensorEngine: bf16 ones[1,128]^T @ emb1[1,512] outer products
    into PSUM, drained to SBUF by the otherwise idle ScalarE. Only a
    16 KB bf16 emb row is read from HBM. bf16 rounding of emb gives rel
    err ~2^-8, far inside the 2e-2 tolerance.
  - Block 0's chunk-0 multiply and store are split into 512-col pieces so
    the first store enters the queue as early as possible.
"""

import numpy as np
import ml_dtypes

import concourse.bass as bass
import concourse.mybir as mybir
from concourse import bacc
from concourse.tile import TileContext
from concourse.bass_utils import run_bass_kernel_spmd

B, S, K = 64, 200, 4
N, D = 128, 64
ND = N * D                      # 8192
NCORES = 8
B_LOC = B // NCORES             # 8
ROWS = B_LOC * S                # 1600 (b,s) rows per core
P = 128
NBLK = (ROWS + P - 1) // P      # 13 (12 full + 1 of 64 rows)

CH = 4                          # emb/mul/store chunks per block
CW = ND // CH                   # 2048 cols per chunk (= 32 n-rows), 1 MB stores
NCH = N // CH                   # 32 n-rows per chunk
MMW = 512                       # matmul moving-dim width (HW max)

_NC_CACHE = {}


def _build_nc():
    nc = bacc.Bacc()
    idx = nc.declare_dram_parameter("idx", [P, NBLK * K], mybir.dt.float32, isOutput=False)
    embone = nc.declare_dram_parameter("embone", [1, ND], mybir.dt.bfloat16, isOutput=False)
    iota = nc.declare_dram_parameter("iota", [P, N], mybir.dt.float32, isOutput=False)
    out = nc.declare_dram_parameter("out", [ROWS, ND], mybir.dt.float32, isOutput=True)

    with TileContext(nc) as tc:
        with (
            tc.tile_pool(name="const", bufs=1) as cpool,
            tc.tile_pool(name="counts", bufs=NBLK) as hpool,
            tc.tile_pool(name="work", bufs=12) as wpool,
            tc.psum_pool(name="psum", bufs=4) as ppool,
        ):
            # emb row first: it feeds the TensorE broadcast of all chunks
            emb1_sb = cpool.tile([1, ND], mybir.dt.bfloat16)
            nc.sync.dma_start(out=emb1_sb, in_=embone[:, :])
            # then the histogram inputs
            iota_sb = cpool.tile([P, N], mybir.dt.float32)
            nc.sync.dma_start(out=iota_sb, in_=iota[:, :])
            idx_sb = cpool.tile([P, NBLK * K], mybir.dt.float32)
            nc.sync.dma_start(out=idx_sb, in_=idx[:, :])

            ones_sb = cpool.tile([1, P], mybir.dt.bfloat16)
            nc.gpsimd.memset(ones_sb, 1.0)

            # 128-partition emb replica built by TensorE outer products,
            # drained PSUM->SBUF by ScalarE. Chunk 0 first: the first
            # multiplies gate on its first 512-col slices.
            emb_sb = cpool.tile([P, ND], mybir.dt.float32)
            for c in range(CH):
                for s in range(CW // MMW):
                    col = c * CW + s * MMW
                    pt = ppool.tile([P, MMW], mybir.dt.float32, tag="pt")
                    nc.tensor.matmul(
                        pt[:, :],
                        lhsT=ones_sb[:, :],
                        rhs=emb1_sb[:, col : col + MMW],
                        start=True,
                        stop=True,
                    )
                    nc.scalar.copy(out=emb_sb[:, col : col + MMW], in_=pt[:, :])

            def emit_hist(j, counts, pj):
                nc.vector.tensor_scalar(
                    out=counts[:pj],
                    in0=iota_sb[:pj],
                    scalar1=idx_sb[:pj, j * K : j * K + 1],
                    scalar2=None,
                    op0=mybir.AluOpType.is_equal,
                )
                for k in range(1, K):
                    nc.vector.scalar_tensor_tensor(
                        out=counts[:pj],
                        in0=iota_sb[:pj],
                        scalar=idx_sb[:pj, j * K + k : j * K + k + 1],
                        in1=counts[:pj],
                        op0=mybir.AluOpType.is_equal,
                        op1=mybir.AluOpType.add,
                    )

            def emit_mul(j, c, counts, pj, split=1):
                ot = wpool.tile([P, CW], mybir.dt.float32, tag="ot")
                w = CW // split
                nw = NCH // split
                for s in range(split):
                    nc.vector.tensor_tensor(
                        out=ot[:pj, s * w : (s + 1) * w].rearrange(
                            "p (n d) -> p n d", d=D
                        ),
                        in0=counts[
                            :pj, c * NCH + s * nw : c * NCH + (s + 1) * nw, None
                        ].broadcast_to([pj, nw, D]),
                        in1=emb_sb[:pj, c * CW + s * w : c * CW + (s + 1) * w].rearrange(
                            "p (n d) -> p n d", d=D
                        ),
                        op=mybir.AluOpType.mult,
                    )
                    nc.sync.dma_start(
                        out=out[
                            j * P : j * P + pj, c * CW + s * w : c * CW + (s + 1) * w
                        ],
                        in_=ot[:pj, s * w : (s + 1) * w],
                    )

            # Blocks are processed in groups of 4: histogram + chunk-0
            # multiply for the group, then its chunks 1..3. Histograms cost
            # DVE time without producing store bytes; grouping spreads them
            # across the whole stream so DVE production (the store
            # producer) never drops below the DMA drain rate for long.
            # Group 0's chunk-1 multiplies start ~21us in, after TensorE
            # has replicated those emb columns (~13us).
            for g in range(0, NBLK, 4):
                blocks = range(g, min(g + 4, NBLK))
                counts_tiles = {}
                for j in blocks:
                    pj = min(P, ROWS - j * P)
                    counts = hpool.tile([P, N], mybir.dt.float32, tag="counts")
                    counts_tiles[j] = counts
                    emit_hist(j, counts, pj)
                    emit_mul(j, 0, counts, pj, split=4 if j == 0 else 1)
                for c in range(1, CH):
                    for j in blocks:
                        pj = min(P, ROWS - j * P)
                        emit_mul(j, c, counts_tiles[j], pj)

    nc.finalize()
    return nc


def _get_nc():
    if "nc" not in _NC_CACHE:
        _NC_CACHE["nc"] = _build_nc()
    return _NC_CACHE["nc"]


def _prepare_in_maps(concepts, emb_table):
    concepts = np.asarray(concepts)
    emb = np.ascontiguousarray(np.asarray(emb_table, dtype=np.float32).reshape(1, ND))

    # per-core index shards, padded to NBLK*P rows, laid out [P, NBLK*K]
    conc = concepts.reshape(NCORES, ROWS, K).astype(np.float32)
    idx_pad = np.full((NCORES, NBLK * P, K), float(N), dtype=np.float32)
    idx_pad[:, :ROWS] = conc
    # [core, NBLK, P, K] -> [core, P, NBLK*K]
    idx_dev = np.ascontiguousarray(
        idx_pad.reshape(NCORES, NBLK, P, K).transpose(0, 2, 1, 3).reshape(NCORES, P, NBLK * K)
    )

    iota = np.ascontiguousarray(
        np.broadcast_to(np.arange(N, dtype=np.float32), (P, N))
    )
    embone = np.ascontiguousarray(emb.astype(ml_dtypes.bfloat16))
    return [
        {"idx": idx_dev[i], "embone": embone, "iota": iota}
        for i in range(NCORES)
    ]


def _run(concepts, emb_table, **spmd_kwargs):
    nc = _get_nc()
    in_maps = _prepare_in_maps(concepts, emb_table)
    res = run_bass_kernel_spmd(nc, in_maps, core_ids=list(range(NCORES)), **spmd_kwargs)
    out = np.concatenate(
        [res.results[i]["out"].reshape(B_LOC, S, N, D) for i in range(NCORES)],
        axis=0,
    )
    return out, res


def kernel(concepts, emb_table):
    out, _ = _run(concepts, emb_table)
    return out
